# revision 25
# baseline (speedup 1.0000x reference)
"""Causal self-attention (B=4, S=2048, D=1024, H=16) on 8 trn2 cores.

Sharding: core c = 2*b + g  (b = batch 0..3, g = head-group 0..1, 8 heads/group).
Each core computes, for its batch element and its 8 heads:
    qkv -> causal attention -> y @ w_proj[rows of its head group]
The two head-group partial outputs per batch are summed on the host.

Device layouts (bf16 matmul operands, fp32 PSUM accumulate):
    xT [D, S]       x[b] transposed on host (contraction dim on partitions)
    wq/wk/wv [D, 512]  w_qkv column slices for the group
    wp [512, D]     w_proj row slice
On-chip: Q^T,K^T in [head_dim, s] layout; scores computed transposed
([sk, sq]) so exp(p) feeds the AV matmul directly as the moving operand;
an all-ones column appended to V gives the softmax denominator for free
(row 64 of the AV psum).  Causal masking is done on the tensor engine:
for each diagonal sk-chunk a second matmul accumulates -1e5 onto the
non-causal triangle (lhsT = strict-upper-tri constant, rhs = shifted
identity band), so exp underflows to exactly 0 there -- no vector/gpsimd
mask pass and no post-exp dependency.  Normalization is deferred to Y:
denominators l are packed [128,8] via a DMA transpose, reciprocated in
one tiny DVE op, and 1/l is broadcast back through DRAM.
"""

import numpy as np
import ml_dtypes

import concourse.mybir as mybir
import concourse.tile as tile
from concourse import bacc
from concourse.bass_utils import run_bass_kernel_spmd

P = 128
D = 1024
KD = D // P          # 8 contraction chunks
GCOLS = 512          # qkv cols per head group
HG = 8               # heads per core
HD = 64
VP = HD + 2          # V row pitch (65 used, padded even for alignment)
NJ = 4               # head-pair col tiles (2 heads x 64 = 128)
SQT = 512            # sq tile (matmul moving dim)
F32 = mybir.dt.float32
BF16 = mybir.dt.bfloat16
NPBF16 = ml_dtypes.bfloat16

TRACE = False
PPOOL_BUFS = 6
SC_BUFS = 3
ST_BUFS = 3
QKT_BUFS = 2
YP_BUFS = 2
WS_BUFS = 2
PJ_BUFS = 9
TRACE_KWARGS = {}


def build_nc(S=2048):
    NT4 = S // SQT       # sq tiles of 512
    NT16 = S // P        # s chunks of 128
    nc = bacc.Bacc("TRN2", target_bir_lowering=False, debug=False)

    xT = nc.dram_tensor("xT", [D, S], BF16, kind="ExternalInput").ap()
    wq = nc.dram_tensor("wq", [D, GCOLS], BF16, kind="ExternalInput").ap()
    wk = nc.dram_tensor("wk", [D, GCOLS], BF16, kind="ExternalInput").ap()
    wv = nc.dram_tensor("wv", [D, GCOLS], BF16, kind="ExternalInput").ap()
    wp = nc.dram_tensor("wp", [GCOLS, D], BF16, kind="ExternalInput").ap()
    mtri = nc.dram_tensor("mtri", [P, P], BF16, kind="ExternalInput").ap()
    eband = nc.dram_tensor("eband", [P, 768], BF16, kind="ExternalInput").ap()
    out = nc.dram_tensor("out", [S, D], F32, kind="ExternalOutput").ap()

    with tile.TileContext(nc) as tc:
        with (
            tc.tile_pool(name="persist", bufs=1) as persist,
            tc.tile_pool(name="qkt", bufs=QKT_BUFS) as qkt,
            tc.tile_pool(name="ppool", bufs=PPOOL_BUFS) as ppool,
            tc.tile_pool(name="stpool", bufs=ST_BUFS) as stpool,
            tc.tile_pool(name="btpool", bufs=ST_BUFS) as btpool,
            tc.tile_pool(name="lppool", bufs=2) as lppool,
            tc.tile_pool(name="dram", bufs=1, space="DRAM") as drampool,
            tc.tile_pool(name="ps_sc", bufs=SC_BUFS, space="PSUM") as ps_sc,
            tc.tile_pool(name="ps_y", bufs=YP_BUFS, space="PSUM") as ps_y,
        ):
            V = persist.tile([P, NT16, HG, VP], BF16)
            MT = persist.tile([P, P], BF16)
            EB = persist.tile([P, 768], BF16)
            WP = persist.tile([P, NJ, D], BF16)
            nc.sync.dma_start(out=MT, in_=mtri)
            nc.sync.dma_start(out=EB, in_=eband)
            nc.sync.dma_start(
                out=WP, in_=wp.rearrange("(j p) d -> p j d", p=P)
            )
            ld = drampool.tile([S // SQT, HG, SQT], BF16)
            ldr = drampool.tile([S // SQT, HG, SQT], BF16)
            yd = drampool.tile([GCOLS, S], BF16)

            qT_sb = {}
            kT_sb = {}
            wq_sb = {}
            wk_sb = {}

            def qk_loads(j):
                wqj = wstream.tile([P, KD, P], BF16, tag="wqj")
                wkj = wstream.tile([P, KD, P], BF16, tag="wkj")
                nc.sync.dma_start(
                    out=wqj,
                    in_=wq[:, j * P : (j + 1) * P].rearrange(
                        "(k p) c -> p k c", p=P
                    ),
                )
                nc.sync.dma_start(
                    out=wkj,
                    in_=wk[:, j * P : (j + 1) * P].rearrange(
                        "(k p) c -> p k c", p=P
                    ),
                )
                wq_sb[j] = wqj
                wk_sb[j] = wkj
                qT_sb[j] = qkt.tile([P, S], BF16, name="qTj", tag="qTj")
                kT_sb[j] = qkt.tile([P, S], BF16, name="kTj", tag="kTj")

            def qk_tile(j, t):
                """Q^T,K^T matmuls for head-pair col-tile j, s-tile t."""
                xTs = qk_tile.xTs
                if j not in wq_sb:
                    qk_loads(j)
                for wsb, dest in (
                    (wq_sb[j], qT_sb[j]),
                    (wk_sb[j], kT_sb[j]),
                ):
                    ps = ps_sc.tile([P, SQT], F32, name="ps_qk", tag="sc")
                    for k in range(KD):
                        nc.tensor.matmul(
                            ps,
                            lhsT=wsb[:, k, :],
                            rhs=xTs[:, k, t * SQT : (t + 1) * SQT],
                            start=(k == 0),
                            stop=(k == KD - 1),
                        )
                    nc.vector.tensor_copy(
                        out=dest[:, t * SQT : (t + 1) * SQT], in_=ps
                    )

            def attn_tile(j, t, fillers=()):
                """Scores+softmax+AV for heads (2j, 2j+1) on sq tile t.

                Software-pipelined: AV of group g is emitted after the
                scores+exp of group g+1 so PE has work while ACT runs."""
                qTj = qT_sb[j]
                kTj = kT_sb[j]
                nch = 4 * t + 4  # causal sk chunks of 128
                yps = {}
                for hi in (0, 1):
                    h = 2 * j + hi
                    yps[h] = ps_y.tile(
                        [HD + 1, SQT], F32, name="yps", tag="yps"
                    )

                def chunk_off(c):
                    # causal column offset within the sq tile
                    if c < 4 * t:
                        return 0
                    return P * (c - 4 * t)

                def emit_scores_exp(g):
                    w = min(2, nch - g)
                    offs = [chunk_off(g + ci) for ci in range(w)]
                    scs = {}
                    for hi in (0, 1):
                        scs[hi] = ps_sc.tile(
                            [P, 2 * SQT], F32, name="sc", tag="sc"
                        )
                    # ABAB emission: the two heads' K=64 matmuls live in
                    # disjoint PE row groups and stream concurrently
                    for ci in range(w):
                        c = g + ci
                        off = offs[ci]
                        diag = c >= 4 * t
                        for hi in (0, 1):
                            base = HD * hi
                            nc.tensor.matmul(
                                scs[hi][:, ci * SQT + off : (ci + 1) * SQT],
                                lhsT=kTj[base : base + HD, c * P : (c + 1) * P],
                                rhs=qTj[
                                    base : base + HD,
                                    t * SQT + off : (t + 1) * SQT,
                                ],
                                start=True,
                                stop=not diag,
                            )
                    for ci in range(w):
                        c = g + ci
                        off = offs[ci]
                        if c >= 4 * t:
                            # -1e5 onto the non-causal triangle: out[i,w]
                            # = MT[w-off, i] = -1e5 where i > w-off; zero
                            # beyond 128 cols, so only stream 128.
                            for hi in (0, 1):
                                nc.tensor.matmul(
                                    scs[hi][:, ci * SQT + off : ci * SQT + off + P],
                                    lhsT=MT,
                                    rhs=EB[:, 256 : 256 + P],
                                    start=False,
                                    stop=True,
                                )
                    pts = {}
                    for hi in (0, 1):
                        h = 2 * j + hi
                        sc = scs[hi]
                        p = ppool.tile([P, 2 * SQT], BF16, name="pexp")
                        if all(o == 0 for o in offs):
                            nc.scalar.activation(
                                out=p[:, : w * SQT],
                                in_=sc[:, : w * SQT],
                                func=mybir.ActivationFunctionType.Exp,
                                scale=0.125,
                            )
                        else:
                            for ci in range(w):
                                off = offs[ci]
                                nc.scalar.activation(
                                    out=p[:, ci * SQT + off : (ci + 1) * SQT],
                                    in_=sc[:, ci * SQT + off : (ci + 1) * SQT],
                                    func=mybir.ActivationFunctionType.Exp,
                                    scale=0.125,
                                )
                        pts[h] = p
                    return pts

                def emit_av(g, pts):
                    w = min(2, nch - g)
                    for hi in (0, 1):
                        h = 2 * j + hi
                        for ci in range(w):
                            c = g + ci
                            off = chunk_off(c)
                            nc.tensor.matmul(
                                yps[h][:, off:SQT],
                                lhsT=V[:, c, h, 0 : HD + 1],
                                rhs=pts[h][:, ci * SQT + off : (ci + 1) * SQT],
                                start=(c == 0),
                                stop=(c == nch - 1),
                            )

                fill_iter = iter(fillers)
                pend = []
                for g in range(0, nch, 2):
                    pts = emit_scores_exp(g)
                    if len(pend) == 2:
                        emit_av(*pend.pop(0))
                    f = next(fill_iter, None)
                    if f is not None:
                        f()
                    pend.append((g, pts))
                for pr in pend:
                    emit_av(*pr)
                for f in fill_iter:
                    f()

                sts = {}
                for hi in (0, 1):
                    h = 2 * j + hi
                    st = stpool.tile([HD + 1, SQT], BF16, name="st")
                    nc.vector.tensor_copy(out=st, in_=yps[h])
                    nc.sync.dma_start(
                        out=ld[t, h : h + 1, :],
                        in_=st[HD : HD + 1, :],
                    )
                    sts[hi] = st
                # pack both heads' denominators [2,512] -> [128,8],
                # reciprocate once, write 1/l back for the broadcast read
                lsl = ld[t, 2 * j : 2 * j + 2, :]
                rsl = ldr[t, 2 * j : 2 * j + 2, :]
                lp = lppool.tile([P, 8], BF16, name="lp")
                nc.sync.dma_start(
                    out=lp, in_=lsl.rearrange("h (a b) -> (h a) b", b=8)
                )
                with nc.allow_low_precision(reason="bf16 1/l"):
                    nc.vector.reciprocal(out=lp, in_=lp)
                nc.sync.dma_start(
                    out=rsl.rearrange("h (a b) -> (h a) b", b=8), in_=lp
                )
                for hi in (0, 1):
                    h = 2 * j + hi
                    st = sts[hi]
                    bt = btpool.tile([HD, SQT], BF16, name="bt")
                    nc.sync.dma_start(
                        out=bt,
                        in_=ldr[t, h : h + 1, :].to_broadcast([HD, SQT]),
                    )
                    nc.gpsimd.tensor_mul(st[0:HD, :], st[0:HD, :], bt)
                    if j == NJ - 1:
                        ydst = attn_tile.y3[
                            HD * hi : HD * (hi + 1), t * SQT : (t + 1) * SQT
                        ]
                    else:
                        ydst = yd[
                            j * P + HD * hi : j * P + HD * (hi + 1),
                            t * SQT : (t + 1) * SQT,
                        ]
                    nc.sync.dma_start(out=ydst, in_=st[0:HD, :])

            with (
                tc.tile_pool(name="qkv_in", bufs=1) as qkv_in,
                tc.tile_pool(name="wstream", bufs=WS_BUFS) as wstream_,
            ):
                wstream = wstream_
                xTs = qkv_in.tile([P, KD, S], BF16)
                wvs = qkv_in.tile([P, KD, GCOLS], BF16)
                qk_tile.xTs = xTs

                # ---- input loads: s-major so the V/QK matmuls for the
                # first sq block start after 1/4 of xT has landed ----
                wvr = wv.rearrange("(k p) c -> k p c", p=P)
                xTr = xT.rearrange("(k p) s -> k p s", p=P)
                for k in range(KD):
                    nc.sync.dma_start(out=wvs[:, k, :], in_=wvr[k])
                    nc.sync.dma_start(
                        out=xTs[:, k, 0:SQT], in_=xTr[k][:, 0:SQT]
                    )
                qk_loads(0)
                for s4 in range(1, NT4):
                    for k in range(KD):
                        nc.sync.dma_start(
                            out=xTs[:, k, s4 * SQT : (s4 + 1) * SQT],
                            in_=xTr[k][:, s4 * SQT : (s4 + 1) * SQT],
                        )
                onesrow = qkv_in.tile([P, NT16 * HG], BF16)
                nc.vector.memset(onesrow, 1.0)
                nc.vector.tensor_copy(
                    out=V[:, :, :, HD : HD + 1],
                    in_=onesrow.rearrange(
                        "p (t h one) -> p t h one", t=NT16, one=1
                    ),
                )

                # ---- V = x @ wv  (natural [s, vcol] layout), interleaved
                # with the Q^T/K^T builds per sq block ----
                def v_tile(t):
                    ps = ps_sc.tile([P, GCOLS], F32, name="ps_v", tag="sc")
                    for k in range(KD):
                        nc.tensor.matmul(
                            ps,
                            lhsT=xTs[:, k, t * P : (t + 1) * P],
                            rhs=wvs[:, k, :],
                            start=(k == 0),
                            stop=(k == KD - 1),
                        )
                    nc.vector.tensor_copy(
                        out=V[:, t, :, 0:HD],
                        in_=ps.rearrange("p (h d) -> p h d", h=HG),
                    )

                for s4 in range(NT4):
                    for t in range(4 * s4, 4 * s4 + 4):
                        v_tile(t)
                    qk_tile(0, s4)
                for j in range(NJ - 1):
                    for t in range(NT4):
                        attn_tile(j, t)
                        qk_tile(j + 1, t)

            # ---- last head-pair + projection, overlapped ----
            with (
                tc.tile_pool(name="late", bufs=1) as late,
                tc.tile_pool(name="projin", bufs=PJ_BUFS) as projin,
                tc.tile_pool(name="outst", bufs=PJ_BUFS) as outst,
            ):
                Y3 = late.tile([P, S], BF16)
                attn_tile.y3 = Y3
                ydr = yd.rearrange("(j p) s -> p j s", p=P)
                yts = {}

                def prefetch_yt(t):
                    yt = projin.tile([P, NJ - 1, P], BF16, name="yt")
                    nc.sync.dma_start(
                        out=yt, in_=ydr[:, 0 : NJ - 1, t * P : (t + 1) * P]
                    )
                    yts[t] = yt

                def proj_tile(t):
                    yt = yts.pop(t)
                    for n in range(D // SQT):
                        pp = ps_sc.tile([P, SQT], F32, name="pp", tag="sc")
                        for j in range(NJ):
                            lhsT = (
                                yt[:, j, :]
                                if j < NJ - 1
                                else Y3[:, t * P : (t + 1) * P]
                            )
                            nc.tensor.matmul(
                                pp,
                                lhsT=lhsT,
                                rhs=WP[:, j, n * SQT : (n + 1) * SQT],
                                start=(j == 0),
                                stop=(j == NJ - 1),
                            )
                        ot = outst.tile([P, SQT], F32, name="ot")
                        nc.vector.tensor_copy(out=ot, in_=pp)
                        nc.sync.dma_start(
                            out=out[t * P : (t + 1) * P, n * SQT : (n + 1) * SQT],
                            in_=ot,
                        )

                for t in range(NT4):
                    if t > 0:
                        for tp in range(4 * (t - 1), 4 * t):
                            prefetch_yt(tp)
                    if t == NT4 - 1:
                        for tp in range(4 * t, 4 * t + 4):
                            prefetch_yt(tp)
                    attn_tile(NJ - 1, t)
                    if t > 0:
                        for tp in range(4 * (t - 1), 4 * t):
                            proj_tile(tp)
                for tp in range(4 * (NT4 - 1), 4 * NT4):
                    proj_tile(tp)
    nc.compile()
    return nc


_NC_CACHE = {}


def _get_nc(S=2048):
    if S not in _NC_CACHE:
        _NC_CACHE[S] = build_nc(S)
    return _NC_CACHE[S]


def make_masks():
    mtri = np.triu(np.full((P, P), -1e5, np.float32), 1).astype(NPBF16)
    eband = np.zeros((P, 768), np.float32)
    eband[np.arange(P), 256 + np.arange(P)] = 1.0
    return mtri, eband.astype(NPBF16)


def shard_inputs(x, w_qkv, w_proj):
    mtri, eband = make_masks()
    w16 = w_qkv.astype(NPBF16)
    wp16 = w_proj.astype(NPBF16)
    ins = []
    for c in range(8):
        b, g = divmod(c, 2)
        ins.append(
            {
                "xT": np.ascontiguousarray(x[b].T.astype(NPBF16)),
                "wq": np.ascontiguousarray(w16[:, g * GCOLS : (g + 1) * GCOLS]),
                "wk": np.ascontiguousarray(
                    w16[:, D + g * GCOLS : D + (g + 1) * GCOLS]
                ),
                "wv": np.ascontiguousarray(
                    w16[:, 2 * D + g * GCOLS : 2 * D + (g + 1) * GCOLS]
                ),
                "wp": np.ascontiguousarray(wp16[g * GCOLS : (g + 1) * GCOLS, :]),
                "mtri": mtri,
                "eband": eband,
            }
        )
    return ins


_LAST_RESULT = None


def kernel(x, w_qkv, w_proj):
    global _LAST_RESULT
    x = np.asarray(x, dtype=np.float32)
    w_qkv = np.asarray(w_qkv, dtype=np.float32)
    w_proj = np.asarray(w_proj, dtype=np.float32)
    S = x.shape[1]
    nc = _get_nc(S)
    ins = shard_inputs(x, w_qkv, w_proj)
    res = run_bass_kernel_spmd(
        nc,
        ins,
        core_ids=list(range(8)),
        trace=TRACE,
        **TRACE_KWARGS,
    )
    _LAST_RESULT = res
    outs = [res.results[c]["out"] for c in range(8)]
    return np.stack([outs[2 * b] + outs[2 * b + 1] for b in range(4)])


# revision 26
# speedup vs baseline: 1.1515x; 1.1515x over previous
"""Causal self-attention (B=4, S=2048, D=1024, H=16) on 8 trn2 cores.

Sharding: core c = 2*b + g  (b = batch 0..3, g = head-group 0..1, 8 heads/group).
Each core computes, for its batch element and its 8 heads:
    qkv -> causal attention -> y @ w_proj[rows of its head group]
The two head-group partial outputs per batch are summed on the host.

Device layouts (bf16 matmul operands, fp32 PSUM accumulate):
    xT [D, S]       x[b] transposed on host (contraction dim on partitions)
    wq/wk/wv [D, 512]  w_qkv column slices for the group
    wp [512, D]     w_proj row slice
On-chip: Q^T,K^T in [head_dim, s] layout; scores computed transposed
([sk, sq]) so exp(p) feeds the AV matmul directly as the moving operand;
an all-ones column appended to V gives the softmax denominator for free
(row 64 of the AV psum).  Causal masking is done on the tensor engine:
for each diagonal sk-chunk a second matmul accumulates -1e5 onto the
non-causal triangle (lhsT = strict-upper-tri constant, rhs = shifted
identity band), so exp underflows to exactly 0 there -- no vector/gpsimd
mask pass and no post-exp dependency.  Normalization is deferred to Y:
denominators l are packed [128,8] via a DMA transpose, reciprocated in
one tiny DVE op, and 1/l is broadcast back through DRAM.
"""

import numpy as np
import ml_dtypes

import concourse.mybir as mybir
import concourse.tile as tile
from concourse import bacc
from concourse.bass_utils import run_bass_kernel_spmd

P = 128
D = 1024
KD = D // P          # 8 contraction chunks
GCOLS = 512          # qkv cols per head group
HG = 8               # heads per core
HD = 64
VP = HD + 2          # V row pitch (65 used, padded even for alignment)
NJ = 4               # head-pair col tiles (2 heads x 64 = 128)
SQT = 512            # sq tile (matmul moving dim)
F32 = mybir.dt.float32
BF16 = mybir.dt.bfloat16
NPBF16 = ml_dtypes.bfloat16

TRACE = False
PPOOL_BUFS = 4
SC_BUFS = 3
ST_BUFS = 4
QKT_BUFS = 2
YP_BUFS = 2
WS_BUFS = 2
PJ_BUFS = 9
TRACE_KWARGS = {}


def build_nc(S=2048):
    NT4 = S // SQT       # sq tiles of 512
    NT16 = S // P        # s chunks of 128
    nc = bacc.Bacc("TRN2", target_bir_lowering=False, debug=False)

    xT = nc.dram_tensor("xT", [D, S], BF16, kind="ExternalInput").ap()
    wq = nc.dram_tensor("wq", [D, GCOLS], BF16, kind="ExternalInput").ap()
    wk = nc.dram_tensor("wk", [D, GCOLS], BF16, kind="ExternalInput").ap()
    wv = nc.dram_tensor("wv", [D, GCOLS], BF16, kind="ExternalInput").ap()
    wp = nc.dram_tensor("wp", [GCOLS, D], BF16, kind="ExternalInput").ap()
    mtri = nc.dram_tensor("mtri", [P, P], BF16, kind="ExternalInput").ap()
    eband = nc.dram_tensor("eband", [P, 768], BF16, kind="ExternalInput").ap()
    out = nc.dram_tensor("out", [S, D], F32, kind="ExternalOutput").ap()

    with tile.TileContext(nc) as tc:
        with (
            tc.tile_pool(name="persist", bufs=1) as persist,
            tc.tile_pool(name="qkt", bufs=QKT_BUFS) as qkt,
            tc.tile_pool(name="ppool", bufs=PPOOL_BUFS) as ppool,
            tc.tile_pool(name="stpool", bufs=ST_BUFS) as stpool,
            tc.tile_pool(name="btpool", bufs=ST_BUFS) as btpool,
            tc.tile_pool(name="lppool", bufs=2) as lppool,
            tc.tile_pool(name="dram", bufs=1, space="DRAM") as drampool,
            tc.tile_pool(name="ps_sc", bufs=SC_BUFS, space="PSUM") as ps_sc,
            tc.tile_pool(name="ps_y", bufs=YP_BUFS, space="PSUM") as ps_y,
        ):
            V = persist.tile([P, NT16, HG, VP], BF16)
            MT = persist.tile([P, P], BF16)
            EB = persist.tile([P, 768], BF16)
            WP = persist.tile([P, NJ, D], BF16)
            nc.sync.dma_start(out=MT, in_=mtri)
            nc.sync.dma_start(out=EB, in_=eband)
            nc.sync.dma_start(
                out=WP, in_=wp.rearrange("(j p) d -> p j d", p=P)
            )
            ld = drampool.tile([S // SQT, HG, SQT], BF16)
            ldr = drampool.tile([S // SQT, HG, SQT], BF16)
            yd = drampool.tile([GCOLS, S], BF16)

            qT_sb = {}
            kT_sb = {}
            wq_sb = {}
            wk_sb = {}

            def qk_loads(j):
                wqj = wstream.tile([P, KD, P], BF16, tag="wqj")
                wkj = wstream.tile([P, KD, P], BF16, tag="wkj")
                nc.sync.dma_start(
                    out=wqj,
                    in_=wq[:, j * P : (j + 1) * P].rearrange(
                        "(k p) c -> p k c", p=P
                    ),
                )
                nc.sync.dma_start(
                    out=wkj,
                    in_=wk[:, j * P : (j + 1) * P].rearrange(
                        "(k p) c -> p k c", p=P
                    ),
                )
                wq_sb[j] = wqj
                wk_sb[j] = wkj
                qT_sb[j] = qkt.tile([P, S], BF16, name="qTj", tag="qTj")
                kT_sb[j] = qkt.tile([P, S], BF16, name="kTj", tag="kTj")

            def qk_tile(j, t):
                """Q^T,K^T matmuls for head-pair col-tile j, s-tile t."""
                xTs = qk_tile.xTs
                if j not in wq_sb:
                    qk_loads(j)
                for wsb, dest in (
                    (wq_sb[j], qT_sb[j]),
                    (wk_sb[j], kT_sb[j]),
                ):
                    ps = ps_sc.tile([P, SQT], F32, name="ps_qk", tag="sc")
                    for k in range(KD):
                        nc.tensor.matmul(
                            ps,
                            lhsT=wsb[:, k, :],
                            rhs=xTs[:, k, t * SQT : (t + 1) * SQT],
                            start=(k == 0),
                            stop=(k == KD - 1),
                        )
                    nc.vector.tensor_copy(
                        out=dest[:, t * SQT : (t + 1) * SQT], in_=ps
                    )

            def attn_tile(j, t, fillers=()):
                """Scores+softmax+AV for heads (2j, 2j+1) on sq tile t.

                Software-pipelined: AV of group g is emitted after the
                scores+exp of group g+1 so PE has work while ACT runs."""
                qTj = qT_sb[j]
                kTj = kT_sb[j]
                nch = 4 * t + 4  # causal sk chunks of 128
                yps = {}
                for hi in (0, 1):
                    h = 2 * j + hi
                    yps[h] = ps_y.tile(
                        [HD + 1, SQT], F32, name="yps", tag="yps"
                    )

                def chunk_off(c):
                    # causal column offset within the sq tile
                    if c < 4 * t:
                        return 0
                    return P * (c - 4 * t)

                def emit_scores_exp(g):
                    w = min(2, nch - g)
                    offs = [chunk_off(g + ci) for ci in range(w)]
                    scs = {}
                    for hi in (0, 1):
                        scs[hi] = ps_sc.tile(
                            [P, 2 * SQT], F32, name="sc", tag="sc"
                        )
                    # ABAB emission: the two heads' K=64 matmuls live in
                    # disjoint PE row groups and stream concurrently
                    for ci in range(w):
                        c = g + ci
                        off = offs[ci]
                        diag = c >= 4 * t
                        for hi in (0, 1):
                            base = HD * hi
                            nc.tensor.matmul(
                                scs[hi][:, ci * SQT + off : (ci + 1) * SQT],
                                lhsT=kTj[base : base + HD, c * P : (c + 1) * P],
                                rhs=qTj[
                                    base : base + HD,
                                    t * SQT + off : (t + 1) * SQT,
                                ],
                                start=True,
                                stop=not diag,
                            )
                    for ci in range(w):
                        c = g + ci
                        off = offs[ci]
                        if c >= 4 * t:
                            # -1e5 onto the non-causal triangle: out[i,w]
                            # = MT[w-off, i] = -1e5 where i > w-off; zero
                            # beyond 128 cols, so only stream 128.
                            for hi in (0, 1):
                                nc.tensor.matmul(
                                    scs[hi][:, ci * SQT + off : ci * SQT + off + P],
                                    lhsT=MT,
                                    rhs=EB[:, 256 : 256 + P],
                                    start=False,
                                    stop=True,
                                )
                    pts = {}
                    for hi in (0, 1):
                        h = 2 * j + hi
                        sc = scs[hi]
                        p = ppool.tile([P, 2 * SQT], BF16, name="pexp")
                        if all(o == 0 for o in offs):
                            nc.scalar.activation(
                                out=p[:, : w * SQT],
                                in_=sc[:, : w * SQT],
                                func=mybir.ActivationFunctionType.Exp,
                                scale=0.125,
                            )
                        else:
                            for ci in range(w):
                                off = offs[ci]
                                nc.scalar.activation(
                                    out=p[:, ci * SQT + off : (ci + 1) * SQT],
                                    in_=sc[:, ci * SQT + off : (ci + 1) * SQT],
                                    func=mybir.ActivationFunctionType.Exp,
                                    scale=0.125,
                                )
                        pts[h] = p
                    return pts

                def emit_av(g, pts):
                    w = min(2, nch - g)
                    for hi in (0, 1):
                        h = 2 * j + hi
                        for ci in range(w):
                            c = g + ci
                            off = chunk_off(c)
                            nc.tensor.matmul(
                                yps[h][:, off:SQT],
                                lhsT=V[:, c, h, 0 : HD + 1],
                                rhs=pts[h][:, ci * SQT + off : (ci + 1) * SQT],
                                start=(c == 0),
                                stop=(c == nch - 1),
                            )

                fill_iter = iter(fillers)
                prev = None
                for g in range(0, nch, 2):
                    pts = emit_scores_exp(g)
                    if prev is not None:
                        emit_av(*prev)
                    f = next(fill_iter, None)
                    if f is not None:
                        f()
                    prev = (g, pts)
                emit_av(*prev)
                for f in fill_iter:
                    f()

                sts = {}
                for hi in (0, 1):
                    h = 2 * j + hi
                    st = stpool.tile([HD + 1, SQT], BF16, name="st")
                    nc.vector.tensor_copy(out=st, in_=yps[h])
                    nc.sync.dma_start(
                        out=ld[t, h : h + 1, :],
                        in_=st[HD : HD + 1, :],
                    )
                    sts[hi] = st
                # pack both heads' denominators [2,512] -> [128,8],
                # reciprocate once, write 1/l back for the broadcast read
                lsl = ld[t, 2 * j : 2 * j + 2, :]
                rsl = ldr[t, 2 * j : 2 * j + 2, :]
                lp = lppool.tile([P, 8], BF16, name="lp")
                nc.sync.dma_start(
                    out=lp, in_=lsl.rearrange("h (a b) -> (h a) b", b=8)
                )
                with nc.allow_low_precision(reason="bf16 1/l"):
                    nc.vector.reciprocal(out=lp, in_=lp)
                nc.sync.dma_start(
                    out=rsl.rearrange("h (a b) -> (h a) b", b=8), in_=lp
                )
                for hi in (0, 1):
                    h = 2 * j + hi
                    st = sts[hi]
                    bt = btpool.tile([HD, SQT], BF16, name="bt")
                    nc.sync.dma_start(
                        out=bt,
                        in_=ldr[t, h : h + 1, :].to_broadcast([HD, SQT]),
                    )
                    nc.vector.tensor_mul(st[0:HD, :], st[0:HD, :], bt)
                    if j == NJ - 1:
                        ydst = attn_tile.y3[
                            HD * hi : HD * (hi + 1), t * SQT : (t + 1) * SQT
                        ]
                    else:
                        ydst = yd[
                            j * P + HD * hi : j * P + HD * (hi + 1),
                            t * SQT : (t + 1) * SQT,
                        ]
                    nc.sync.dma_start(out=ydst, in_=st[0:HD, :])

            with (
                tc.tile_pool(name="qkv_in", bufs=1) as qkv_in,
                tc.tile_pool(name="wstream", bufs=WS_BUFS) as wstream_,
            ):
                wstream = wstream_
                xTs = qkv_in.tile([P, KD, S], BF16)
                wvs = qkv_in.tile([P, KD, GCOLS], BF16)
                qk_tile.xTs = xTs

                # ---- input loads: s-major so the V/QK matmuls for the
                # first sq block start after 1/4 of xT has landed ----
                wvr = wv.rearrange("(k p) c -> k p c", p=P)
                xTr = xT.rearrange("(k p) s -> k p s", p=P)
                for k in range(KD):
                    nc.sync.dma_start(out=wvs[:, k, :], in_=wvr[k])
                    nc.sync.dma_start(
                        out=xTs[:, k, 0:SQT], in_=xTr[k][:, 0:SQT]
                    )
                qk_loads(0)
                for s4 in range(1, NT4):
                    for k in range(KD):
                        nc.sync.dma_start(
                            out=xTs[:, k, s4 * SQT : (s4 + 1) * SQT],
                            in_=xTr[k][:, s4 * SQT : (s4 + 1) * SQT],
                        )
                onesrow = qkv_in.tile([P, NT16 * HG], BF16)
                nc.vector.memset(onesrow, 1.0)
                nc.vector.tensor_copy(
                    out=V[:, :, :, HD : HD + 1],
                    in_=onesrow.rearrange(
                        "p (t h one) -> p t h one", t=NT16, one=1
                    ),
                )

                # ---- V = x @ wv  (natural [s, vcol] layout), interleaved
                # with the Q^T/K^T builds per sq block ----
                def v_tile(t):
                    ps = ps_sc.tile([P, GCOLS], F32, name="ps_v", tag="sc")
                    for k in range(KD):
                        nc.tensor.matmul(
                            ps,
                            lhsT=xTs[:, k, t * P : (t + 1) * P],
                            rhs=wvs[:, k, :],
                            start=(k == 0),
                            stop=(k == KD - 1),
                        )
                    nc.vector.tensor_copy(
                        out=V[:, t, :, 0:HD],
                        in_=ps.rearrange("p (h d) -> p h d", h=HG),
                    )

                for s4 in range(NT4):
                    for t in range(4 * s4, 4 * s4 + 4):
                        v_tile(t)
                    qk_tile(0, s4)
                for j in range(NJ - 1):
                    for t in range(NT4):
                        attn_tile(j, t)
                        qk_tile(j + 1, t)

            # ---- last head-pair + projection, overlapped ----
            with (
                tc.tile_pool(name="late", bufs=1) as late,
                tc.tile_pool(name="projin", bufs=PJ_BUFS) as projin,
                tc.tile_pool(name="outst", bufs=PJ_BUFS) as outst,
            ):
                Y3 = late.tile([P, S], BF16)
                attn_tile.y3 = Y3
                ydr = yd.rearrange("(j p) s -> p j s", p=P)
                yts = {}

                def prefetch_yt(t):
                    yt = projin.tile([P, NJ - 1, P], BF16, name="yt")
                    nc.sync.dma_start(
                        out=yt, in_=ydr[:, 0 : NJ - 1, t * P : (t + 1) * P]
                    )
                    yts[t] = yt

                def proj_tile(t):
                    yt = yts.pop(t)
                    for n in range(D // SQT):
                        pp = ps_sc.tile([P, SQT], F32, name="pp", tag="sc")
                        for j in range(NJ):
                            lhsT = (
                                yt[:, j, :]
                                if j < NJ - 1
                                else Y3[:, t * P : (t + 1) * P]
                            )
                            nc.tensor.matmul(
                                pp,
                                lhsT=lhsT,
                                rhs=WP[:, j, n * SQT : (n + 1) * SQT],
                                start=(j == 0),
                                stop=(j == NJ - 1),
                            )
                        ot = outst.tile([P, SQT], F32, name="ot")
                        nc.vector.tensor_copy(out=ot, in_=pp)
                        nc.sync.dma_start(
                            out=out[t * P : (t + 1) * P, n * SQT : (n + 1) * SQT],
                            in_=ot,
                        )

                for t in range(NT4):
                    if t > 0:
                        for tp in range(4 * (t - 1), 4 * t):
                            prefetch_yt(tp)
                    if t == NT4 - 1:
                        for tp in range(4 * t, 4 * t + 4):
                            prefetch_yt(tp)
                    attn_tile(NJ - 1, t)
                    if t > 0:
                        for tp in range(4 * (t - 1), 4 * t):
                            proj_tile(tp)
                for tp in range(4 * (NT4 - 1), 4 * NT4):
                    proj_tile(tp)
    nc.compile()
    return nc


_NC_CACHE = {}


def _get_nc(S=2048):
    if S not in _NC_CACHE:
        _NC_CACHE[S] = build_nc(S)
    return _NC_CACHE[S]


def make_masks():
    mtri = np.triu(np.full((P, P), -1e5, np.float32), 1).astype(NPBF16)
    eband = np.zeros((P, 768), np.float32)
    eband[np.arange(P), 256 + np.arange(P)] = 1.0
    return mtri, eband.astype(NPBF16)


def shard_inputs(x, w_qkv, w_proj):
    mtri, eband = make_masks()
    w16 = w_qkv.astype(NPBF16)
    wp16 = w_proj.astype(NPBF16)
    ins = []
    for c in range(8):
        b, g = divmod(c, 2)
        ins.append(
            {
                "xT": np.ascontiguousarray(x[b].T.astype(NPBF16)),
                "wq": np.ascontiguousarray(w16[:, g * GCOLS : (g + 1) * GCOLS]),
                "wk": np.ascontiguousarray(
                    w16[:, D + g * GCOLS : D + (g + 1) * GCOLS]
                ),
                "wv": np.ascontiguousarray(
                    w16[:, 2 * D + g * GCOLS : 2 * D + (g + 1) * GCOLS]
                ),
                "wp": np.ascontiguousarray(wp16[g * GCOLS : (g + 1) * GCOLS, :]),
                "mtri": mtri,
                "eband": eband,
            }
        )
    return ins


_LAST_RESULT = None


def kernel(x, w_qkv, w_proj):
    global _LAST_RESULT
    x = np.asarray(x, dtype=np.float32)
    w_qkv = np.asarray(w_qkv, dtype=np.float32)
    w_proj = np.asarray(w_proj, dtype=np.float32)
    S = x.shape[1]
    nc = _get_nc(S)
    ins = shard_inputs(x, w_qkv, w_proj)
    res = run_bass_kernel_spmd(
        nc,
        ins,
        core_ids=list(range(8)),
        trace=TRACE,
        **TRACE_KWARGS,
    )
    _LAST_RESULT = res
    outs = [res.results[c]["out"] for c in range(8)]
    return np.stack([outs[2 * b] + outs[2 * b + 1] for b in range(4)])
